# revision 36
# baseline (speedup 1.0000x reference)
"""Trainium2 Bass kernel for a 2-layer GCN (GRACE encoder) on 8 NeuronCores.

Math (per layer, from the reference):
    h   = Z @ W
    deg = bincount(dst)            (self-loops included in edge list)
    dinv = deg^-1/2
    out = PReLU(segment_sum(h[src] * dinv[src] * dinv[dst], dst) + b)

We use dinv[s]*h[s] = ((dinv*Z) @ W)[s] =: P[s], so the per-edge work is a
pure row-gather of P plus a segment-sum, and all scaling is per-node:
    out = PReLU(dinv * segment_sum(P[src], dst) + b)

Sharding: dst-partitioned. Core c owns dst rows [c*12544, (c+1)*12544).
Each core computes P for its own rows, an AllGather makes the full P table
visible everywhere, and the scatter (segment-sum) is done with one-hot
selection matmuls accumulating in PSUM, 128 edges per matmul.

Tables travel as f16 (P2 zero-padded to 128 features so every gathered row
is 256 B). Edge messages are fetched with batched `dma_gather` custom DMA
instructions — int16 indices limit each gather to a 32768-row quarter of
the table, so each dst block does 4 gathers (edges are quarter-sorted).
PSUM accumulation and the PReLU epilogue stay f32.
"""

import os
import sys

for p in ("/opt/trn_rl_repo", "/opt/trn_rl_repo/concourse"):
    if p not in sys.path:
        sys.path.insert(0, p)

# NTFF tracing needs antenv.axon_hooks, absent in this axon client — a stray
# BASS_TRACE=1 in the environment would crash run_bass_kernel_spmd otherwise.
os.environ.setdefault("BASS_NEVER_TRACE", "1")

import numpy as np

import jax

# Persistent XLA compilation cache: repeated kernel() calls re-trace a fresh
# closure inside run_bass_kernel_spmd; the disk cache turns the per-call
# backend compile (XLA + walrus NEFF wrap) into a hash lookup.
jax.config.update("jax_compilation_cache_dir", "/tmp/jax_comp_cache_gcn")
jax.config.update("jax_persistent_cache_min_entry_size_bytes", 0)
jax.config.update("jax_persistent_cache_min_compile_time_secs", 0)

import concourse.bacc as bacc
import concourse.tile as tile
from concourse import bass2jax as _b2j
from concourse import mybir
from concourse.bass import ds
from concourse.bass_utils import run_bass_kernel_spmd
from concourse.masks import make_identity

# --- memoize the PJRT execution path per Bass program -----------------------
# run_bass_via_pjrt builds a fresh closure + jax.jit per call, so every
# kernel() invocation re-loads the executable on device (~0.3 s first-run
# cost). Caching the jitted function per `nc` keeps the loaded executable
# alive across calls; inputs/outputs still transfer and execute every call.
_orig_run_bass_via_pjrt = _b2j.run_bass_via_pjrt
_pjrt_cache = {}


def _cached_run_bass_via_pjrt(nc, in_maps, n_cores):
    if nc.dbg_addr is not None or n_cores <= 1:
        return _orig_run_bass_via_pjrt(nc, in_maps, n_cores)
    ent = _pjrt_cache.get(id(nc))
    if ent is None:
        from jax.experimental.shard_map import shard_map
        from jax.sharding import Mesh, PartitionSpec

        _b2j.install_neuronx_cc_hook()
        partition_name = (nc.partition_id_tensor.name
                          if nc.partition_id_tensor else None)
        in_names, out_names, out_avals = [], [], []
        for alloc in nc.m.functions[0].allocations:
            if not isinstance(alloc, mybir.MemoryLocationSet):
                continue
            name = alloc.memorylocations[0].name
            if alloc.kind == "ExternalInput":
                if name != partition_name:
                    in_names.append(name)
            elif alloc.kind == "ExternalOutput":
                out_names.append(name)
                out_avals.append(jax.core.ShapedArray(
                    tuple(alloc.tensor_shape), mybir.dt.np(alloc.dtype)))
        n_params = len(in_names)
        n_outs = len(out_avals)
        all_names = in_names + out_names
        if partition_name is not None:
            all_names.append(partition_name)

        def _body(*args):
            operands = list(args)
            if partition_name is not None:
                operands.append(_b2j.partition_id_tensor())
            return tuple(_b2j._bass_exec_p.bind(
                *operands, out_avals=tuple(out_avals),
                in_names=tuple(all_names), out_names=tuple(out_names),
                lowering_input_output_aliases=(),
                sim_require_finite=True, sim_require_nnan=True, nc=nc))

        devices = jax.devices()[:n_cores]
        mesh = Mesh(np.asarray(devices), ("core",))
        sharded = jax.jit(
            shard_map(_body, mesh=mesh,
                      in_specs=(PartitionSpec("core"),) * (n_params + n_outs),
                      out_specs=(PartitionSpec("core"),) * n_outs,
                      check_rep=False),
            donate_argnums=tuple(range(n_params, n_params + n_outs)),
            keep_unused=True)
        ent = (sharded, in_names, out_names, out_avals, n_params)
        _pjrt_cache[id(nc)] = ent

    sharded, in_names, out_names, out_avals, n_params = ent
    concat_in = [
        np.concatenate([np.asarray(m[name]) for m in in_maps], axis=0)
        for name in in_names]
    concat_zeros = [
        np.zeros((n_cores * a.shape[0], *a.shape[1:]), a.dtype)
        for a in out_avals]
    out_arrs = sharded(*concat_in, *concat_zeros)
    return [
        {name: np.asarray(out_arrs[i]).reshape(n_cores, *out_avals[i].shape)[c]
         for i, name in enumerate(out_names)}
        for c in range(n_cores)]


_b2j.run_bass_via_pjrt = _cached_run_bass_via_pjrt
# ---------------------------------------------------------------------------

N = 100000
E = 1600000
FIN = 128
HID = 128
FOUT = 64
NCORES = 8
BPC = 12544          # dst rows per core (padded); 8 * 12544 = 100352
NPAD = NCORES * BPC
NBLK = BPC // 128    # 98 dst blocks of 128 per core
PCH = 128            # edges per matmul chunk
NQ = 4               # source quarters (int16 gather index limit)
QS = 32768           # rows per quarter (pow2 -> shift/mask indexing)
QROWS = (32768, 32768, 32768, NPAD - 3 * 32768)   # last quarter is short

F16 = mybir.dt.float16
F32 = mybir.dt.float32
U8 = mybir.dt.uint8
I16 = mybir.dt.int16

_cache = {}


def _preprocess(edge_index):
    """Sort edges by (dst block, src); group per (dst block, src quarter)
    with per-quarter chunk counts Kq (SPMD + loop-friendly). Vectorized."""
    loops = np.arange(N, dtype=np.int32)
    src = np.concatenate([edge_index[0], loops])
    dst = np.concatenate([edge_index[1], loops])
    deg = np.bincount(dst, minlength=N).astype(np.float32)
    dinv = np.zeros(NPAD, np.float32)
    dinv[:N] = np.where(deg > 0, 1.0 / np.sqrt(deg), 0.0)

    blk = dst >> 7                              # global 128-row dst block id
    nblk_glob = NPAD // 128                     # 784
    # group key (dst block, src quarter) fits int16 -> 8x faster radix argsort;
    # within-group edge order is irrelevant (the scatter-sum is commutative)
    g4u = (blk << 2) | (src >> 15)
    order = np.argsort(g4u.astype(np.int16), kind="stable")
    g4 = g4u[order]
    src_s = src[order]
    ld_s = (dst & 127).astype(np.uint8)[order]
    loc_s = (src_s & (QS - 1)).astype(np.int16)

    counts4 = np.bincount(g4, minlength=nblk_glob * NQ)
    cmax = counts4.reshape(nblk_glob, NQ).max(axis=0)
    Kq = tuple(max(1, int(np.ceil(c / PCH))) for c in cmax)
    offq = np.zeros(NQ, np.int32)
    offq[1:] = np.cumsum(Kq)[:-1]
    CB = int(sum(Kq))                           # chunks per dst block
    C = NBLK * CB                               # chunks per core per layer

    bstart = np.zeros(nblk_glob * NQ, np.int32)
    bstart[1:] = np.cumsum(counts4)[:-1]
    # per-group slot base folded into one LUT: pos = lut[g4] + edge index
    gg = np.arange(nblk_glob * NQ, dtype=np.int32)
    gb = gg >> 2
    lut = ((gb // NBLK) * (C * PCH)
           + ((gb - (gb // NBLK) * NBLK) * CB + offq[gg & 3]) * PCH
           - bstart)
    ne = len(src_s)
    pos = lut[g4] + np.arange(ne, dtype=np.int32)

    sa = np.zeros(NCORES * C * PCH, np.int16)
    la = np.full(NCORES * C * PCH, 255, np.uint8)
    sa[pos] = loc_s
    la[pos] = ld_s

    # idxs: per gather call (j,qr) of n=Kq[qr]*128 edges, elem i -> [i%16, i//16]
    idxs_dev = np.empty((NCORES, 16, C * 8), np.int16)
    A = sa.reshape(NCORES, NBLK, CB * PCH)
    out4 = idxs_dev.reshape(NCORES, 16, NBLK, CB * 8)
    for qr in range(NQ):
        o, k = int(offq[qr]), Kq[qr]
        seg = (A[:, :, o * PCH:(o + k) * PCH]
               .reshape(NCORES, NBLK, k * 8, 16).transpose(0, 3, 1, 2))
        out4[:, :, :, o * 8:(o + k) * 8] = seg
    # ldst: [core, C, 128] -> [core, 128, C] (partition = slot in chunk)
    ldst_dev = np.ascontiguousarray(
        la.reshape(NCORES, C, PCH).swapaxes(1, 2))

    return dinv, idxs_dev, ldst_dev, Kq, C


def _build(Kq, a_val):
    """Build the SPMD Bass program (identical on all cores)."""
    CB = int(sum(Kq))
    C = NBLK * CB
    offq = [0] * NQ
    for qr in range(1, NQ):
        offq[qr] = offq[qr - 1] + Kq[qr - 1]
    nc = bacc.Bacc("TRN2", target_bir_lowering=False, debug=False,
                   num_devices=NCORES)

    # inputs are consolidated by dtype — each ExternalInput is one axon
    # transfer, and small transfers pay a fixed per-array cost
    xR = nc.dram_tensor("xR", [BPC, 128], F16, kind="ExternalInput")  # row-major
    idxs = nc.dram_tensor("idxs", [16, C * 8], I16, kind="ExternalInput")
    pk8 = nc.dram_tensor("pk8", [128, C + 128], U8, kind="ExternalInput")
    pk16 = nc.dram_tensor("pk16", [128, HID + 128], F16, kind="ExternalInput")
    pk32 = nc.dram_tensor("pk32", [128, HID + FOUT + NBLK], F32,
                          kind="ExternalInput")
    out = nc.dram_tensor("out", [BPC, FOUT], F16, kind="ExternalOutput")

    P1_my = nc.dram_tensor("P1_my", [BPC, HID], F16, kind="Internal")
    P1_full = nc.dram_tensor("P1_full", [NPAD, HID], F16, kind="Internal")
    P2_my = nc.dram_tensor("P2_my", [BPC, 128], F16, kind="Internal")
    P2_full = nc.dram_tensor("P2_full", [NPAD, 128], F16, kind="Internal")

    with tile.TileContext(nc) as tc:
        with (
            tc.tile_pool(name="persist", bufs=1) as pp,
            tc.tile_pool(name="work", bufs=4) as wp,
            tc.tile_pool(name="gath", bufs=4) as gp,
            tc.tile_pool(name="psA", bufs=2, space="PSUM") as psA,
            tc.tile_pool(name="psB", bufs=2, space="PSUM") as psB,
        ):
            # ---- persistent SBUF state ----
            xT_sb = pp.tile([128, BPC], F16)
            nc.sync.dma_start(out=xT_sb[:], in_=xR[:], transpose=True)
            idx_sb = pp.tile([128, C * 8], I16)
            nc.sync.dma_start(out=idx_sb[0:16, :], in_=idxs[:])
            # replicate the 16-partition wrap across all 8 gpsimd core groups
            nc.sync.dma_start(out=idx_sb[16:32, :], in_=idx_sb[0:16, :])
            nc.sync.dma_start(out=idx_sb[32:64, :], in_=idx_sb[0:32, :])
            nc.sync.dma_start(out=idx_sb[64:128, :], in_=idx_sb[0:64, :])
            pk8_sb = pp.tile([128, C + 128], U8)
            nc.sync.dma_start(out=pk8_sb[:], in_=pk8[:])
            pk16_sb = pp.tile([128, HID + 128], F16)
            nc.sync.dma_start(out=pk16_sb[:], in_=pk16[:])
            pk32_sb = pp.tile([128, HID + FOUT + NBLK], F32)
            nc.sync.dma_start(out=pk32_sb[:], in_=pk32[:])
            ldst_sb = pk8_sb[:, :C]
            iota_sb = pk8_sb[:, C:C + 128]
            W1_sb = pk16_sb[:, :HID]
            W2_sb = pk16_sb[:, HID:HID + 128]
            b1_sb = pk32_sb[:, :HID]
            b2_sb = pk32_sb[:, HID:HID + FOUT]
            dinv_sb = pk32_sb[:, HID + FOUT:]
            ident_sb = pp.tile([128, 128], F16)
            make_identity(nc, ident_sb[:])
            h1T_sb = pp.tile([128, BPC], F16)   # transposed layer-1 output

            # ---- phase A: P1 = dinv * (x @ W1), own shard ----
            # (python-unrolled: matmul lhsT cannot take a register offset)
            for j in range(NBLK):
                ps = psA.tile([128, HID], F32, tag="pcomp")
                nc.tensor.matmul(out=ps[:], lhsT=xT_sb[:, j * 128:(j + 1) * 128],
                                 rhs=W1_sb[:], start=True, stop=True)
                p1t = wp.tile([128, HID], F16, tag="ptile")
                nc.vector.tensor_scalar_mul(p1t[:], ps[:], dinv_sb[:, j:j + 1])
                nc.sync.dma_start(out=P1_my[j * 128:(j + 1) * 128, :], in_=p1t[:])

            nc.gpsimd.collective_compute(
                "AllGather", mybir.AluOpType.bypass,
                replica_groups=[list(range(NCORES))],
                ins=[P1_my[:]], outs=[P1_full[:]],
            )

            def gather_block(i, table, msgtag):
                msg = gp.tile([128, CB * 128], F16, tag=msgtag)
                for qr in range(NQ):
                    k = Kq[qr]
                    o = offq[qr]
                    nc.gpsimd.dma_gather(
                        out_ap=msg[:, o * 128:(o + k) * 128]
                            .rearrange("p (a b) -> p a b", a=k),
                        in_ap=table[qr * QS:qr * QS + QROWS[qr], :],
                        idxs_ap=idx_sb[:, ds(i * (CB * 8) + o * 8, k * 8)],
                        num_idxs=k * 128,
                        num_idxs_reg=k * 128,
                        elem_size=128,
                    )
                return msg

            def sel_block(i):
                selg = wp.tile([128, CB * 128], F16, tag="selg")
                nc.vector.tensor_tensor(
                    out=selg[:].rearrange("p (a b) -> p a b", a=CB),
                    in0=ldst_sb[:, ds(i * CB, CB), None]
                        .to_broadcast([128, CB, 128]),
                    in1=iota_sb[:, None, :].to_broadcast([128, CB, 128]),
                    op=mybir.AluOpType.is_equal)
                return selg

            # ---- phase B: layer-1 gather + scatter matmuls ----
            with tc.For_i(0, NBLK) as i:
                selg = sel_block(i)
                msg = gather_block(i, P1_full, "msg1")
                agg = psA.tile([128, HID], F32, tag="agg")
                for q in range(CB):
                    nc.tensor.matmul(out=agg[:], lhsT=selg[:, q * 128:(q + 1) * 128],
                                     rhs=msg[:, q * 128:q * 128 + HID],
                                     start=(q == 0), stop=(q == CB - 1))
                # finalize: h1 = PReLU(dinv*agg + b1)
                z = wp.tile([128, HID], F32, tag="z1")
                nc.vector.tensor_scalar_mul(z[:], agg[:], dinv_sb[:, ds(i, 1)])
                nc.vector.tensor_tensor(out=z[:], in0=z[:], in1=b1_sb[:],
                                        op=mybir.AluOpType.add)
                za = wp.tile([128, HID], F32, tag="za1")
                nc.vector.tensor_scalar_mul(za[:], z[:], float(a_val))
                h1 = wp.tile([128, HID], F16, tag="h1")
                nc.vector.tensor_tensor(out=h1[:], in0=z[:], in1=za[:],
                                        op=mybir.AluOpType.max)
                # transpose for the layer-2 P matmul
                pt = psB.tile([128, 128], F16, tag="tpose")
                nc.tensor.transpose(out=pt[:], in_=h1[:], identity=ident_sb[:])
                nc.vector.tensor_copy(h1T_sb[:, ds(i * 128, 128)], pt[:])

            # ---- phase C: P2 = dinv * (h1 @ W2pad), own shard ----
            # (python-unrolled: matmul lhsT cannot take a register offset)
            for j in range(NBLK):
                ps = psA.tile([128, 128], F32, tag="pcomp")
                nc.tensor.matmul(out=ps[:], lhsT=h1T_sb[:, j * 128:(j + 1) * 128],
                                 rhs=W2_sb[:], start=True, stop=True)
                p2t = wp.tile([128, 128], F16, tag="ptile")
                nc.vector.tensor_scalar_mul(p2t[:], ps[:], dinv_sb[:, j:j + 1])
                nc.sync.dma_start(out=P2_my[j * 128:(j + 1) * 128, :], in_=p2t[:])

            nc.gpsimd.collective_compute(
                "AllGather", mybir.AluOpType.bypass,
                replica_groups=[list(range(NCORES))],
                ins=[P2_my[:]], outs=[P2_full[:]],
            )

            # ---- phase D: layer-2 gather + scatter + finalize ----
            with tc.For_i(0, NBLK) as i:
                selg = sel_block(i)
                msg = gather_block(i, P2_full, "msg2")
                agg = psA.tile([128, FOUT], F32, tag="agg")
                for q in range(CB):
                    nc.tensor.matmul(out=agg[:], lhsT=selg[:, q * 128:(q + 1) * 128],
                                     rhs=msg[:, q * 128:q * 128 + FOUT],
                                     start=(q == 0), stop=(q == CB - 1))
                z = wp.tile([128, FOUT], F32, tag="z2")
                nc.vector.tensor_scalar_mul(z[:], agg[:], dinv_sb[:, ds(i, 1)])
                nc.vector.tensor_tensor(out=z[:], in0=z[:], in1=b2_sb[:],
                                        op=mybir.AluOpType.add)
                za = wp.tile([128, FOUT], F32, tag="za2")
                nc.vector.tensor_scalar_mul(za[:], z[:], float(a_val))
                yo = wp.tile([128, FOUT], F16, tag="yo")
                nc.vector.tensor_tensor(out=yo[:], in0=z[:], in1=za[:],
                                        op=mybir.AluOpType.max)
                nc.sync.dma_start(out=out[ds(i * 128, 128), :], in_=yo[:])

    nc.compile()
    return nc


def _stage_x(x, W1, b1, W2, b2):
    """Edge-independent input staging (row-major x; device transposes).
    x f32->f16 conversion is split across threads (numpy releases the GIL)."""
    import concurrent.futures as cf

    x_pad = np.empty((NPAD, FIN), np.float16)
    x_pad[N:] = 0
    nth = 4
    bounds = [(i * N // nth, (i + 1) * N // nth) for i in range(nth)]
    with cf.ThreadPoolExecutor(nth) as ex:
        list(ex.map(lambda b: x_pad[b[0]:b[1]].__setitem__(
            slice(None), x[b[0]:b[1]]), bounds))
    xT_list = [x_pad[c * BPC:(c + 1) * BPC] for c in range(NCORES)]

    pk16 = np.zeros((128, HID + 128), np.float16)
    pk16[:, :HID] = W1
    pk16[:, HID:HID + FOUT] = W2
    pk32 = np.empty((128, HID + FOUT + NBLK), np.float32)
    pk32[:, :HID] = b1
    pk32[:, HID:HID + FOUT] = b2
    iota_np = np.tile(np.arange(128, dtype=np.uint8), (128, 1))
    return xT_list, pk16, pk32, iota_np


def kernel(x, edge_index, W1, b1, W2, b2, a, _want_results=False, _trace=False):
    x = np.asarray(x, np.float32)
    edge_index = np.asarray(edge_index, np.int32)
    dinv, idxs_dev, ldst_dev, Kq, C = _preprocess(edge_index)
    xT_list, pk16, pk32b, iota_np = _stage_x(
        x, np.asarray(W1, np.float32), np.asarray(b1, np.float32),
        np.asarray(W2, np.float32), np.asarray(b2, np.float32))

    key = (Kq, float(a))
    if key not in _cache:
        _cache[key] = _build(Kq, float(a))
    nc = _cache[key]
    in_maps = []
    for c in range(NCORES):
        lo, hi = c * BPC, (c + 1) * BPC
        pk8c = np.empty((128, C + 128), np.uint8)
        pk8c[:, :C] = ldst_dev[c]
        pk8c[:, C:] = iota_np
        pk32c = pk32b.copy()
        pk32c[:, HID + FOUT:] = dinv[lo:hi].reshape(NBLK, 128).T
        in_maps.append({
            "xR": xT_list[c],
            "idxs": idxs_dev[c],
            "pk8": pk8c,
            "pk16": pk16,
            "pk32": pk32c,
        })
    try:
        res = run_bass_kernel_spmd(nc, in_maps, core_ids=list(range(NCORES)),
                                   trace=_trace)
    except Exception:
        # transient axon transport/device hiccup — the call is stateless,
        # one retry is safe
        res = run_bass_kernel_spmd(nc, in_maps, core_ids=list(range(NCORES)),
                                   trace=_trace)
    outs = [res.results[c]["out"] for c in range(NCORES)]
    full = np.concatenate(outs, axis=0)[:N].astype(np.float32)
    if _want_results:
        return full, res
    return full
